# revision 10
# baseline (speedup 1.0000x reference)
"""Causal self-attention (B=2, T=2048, C=1024, H=16, Dh=64) on 8 trn2 NeuronCores.

Sharding: 2-way data-parallel over batch x 4-way tensor-parallel over heads.
Core c handles batch b=c//4 and heads 4g..4g+3 where g=c%4:
  - computes q,k (transposed layout) and v for its 4 heads,
  - causal flash-style attention per head entirely on-chip,
  - row-parallel output projection against w_proj[:, g*256:(g+1)*256],
  - returns the partial [T, C] projection in fp16; host sums the 4 partials.

Structure (v2, restructured for engine balance):
  A:  qk blocks m=0,2 (q/k of heads 0,1), k-outer so matmuls stream while
      the x chunks are still arriving from HBM.
  A2: qk blocks m=1,3.
  V:  v projection, k-outer, 16 row-tile accumulators live in PSUM.
  attn: j-outer (q-chunk), i-inner (k-tile); per (i,j) step all 4 heads:
      - scores as 2 row-tiled concurrent K=64 matmul pairs (heads share the
        PE array top/bottom half via tile_position inference),
      - causal mask added on the diagonal as a N=128 band matmul,
      - ONE exp per step over all 4 heads' scores ([128,4,512] PSUM AP),
      - PV with M=65 stationary (64 v-dims + ones row for the denominator).
      Normalization: copy otp->SBUF (frees PSUM fast), reciprocal on DVE,
      partition_broadcast + multiply on GpSimd.
  OP: output projection from otj (stationary) against wpT, fp16 out.

Softmax skips the max-subtraction (scores are O(1) here); the denominator
comes free as a 65th row in the PV matmul's stationary operand.
"""

import numpy as np
import ml_dtypes
from contextlib import ExitStack

import concourse.bass as bass
import concourse.tile as tile
from concourse import bacc, mybir, bass_utils

F32 = mybir.dt.float32
F16 = mybir.dt.float16
BF16 = mybir.dt.bfloat16

T = 2048
C = 1024
HL = 4  # local heads per core
DH = 64
NKT = T // 128  # 16 k-tiles
NQ = T // 512  # 4 q-chunks
NCC = C // 128  # 8 contraction chunks


def _pin_act_table():
    """Restrict the activation-table registry to the single set containing
    both Exp and Ln, so Exp interleaving never reloads tables."""
    import concourse.bacc as bacc_mod
    from concourse.hw_specs import get_activation_tables as real

    def only_combined(arch):
        t = real(arch)
        name = "natural_log_exp_and_others"
        if name in t:
            return {name: t[name]}
        return t

    bacc_mod.get_activation_tables = only_combined


def build_nc():
    nc = bacc.Bacc("TRN2", target_bir_lowering=False, debug=False)
    xt_d = nc.dram_tensor("xt", [C, T], BF16, kind="ExternalInput").ap()
    wqkt_d = nc.dram_tensor("wqkt", [C, 512], BF16, kind="ExternalInput").ap()
    wvt_d = nc.dram_tensor("wvt", [C, 256], BF16, kind="ExternalInput").ap()
    wpt_d = nc.dram_tensor("wpt", [256, C], BF16, kind="ExternalInput").ap()
    p_d = nc.dram_tensor("p", [T, C], F16, kind="ExternalOutput").ap()

    with tile.TileContext(nc) as tc:
        with ExitStack() as ctx:
            _body(ctx, tc, xt_d, wqkt_d, wvt_d, wpt_d, p_d)
    nc.compile()
    return nc


def _body(ctx, tc, xt_d, wqkt_d, wvt_d, wpt_d, p_d):
    nc = tc.nc
    Exp = mybir.ActivationFunctionType.Exp

    persist = ctx.enter_context(tc.tile_pool(name="persist", bufs=1))
    ptp = ctx.enter_context(tc.tile_pool(name="ptp", bufs=3))
    octp = ctx.enter_context(tc.tile_pool(name="octp", bufs=2))
    rrp = ctx.enter_context(tc.tile_pool(name="rrp", bufs=2))
    pout = ctx.enter_context(tc.tile_pool(name="pout", bufs=3))
    pp = ctx.enter_context(tc.tile_pool(name="pp", bufs=2, space="PSUM"))

    # ---- persistent SBUF tiles ----
    identb = persist.tile([128, 128], BF16, tag="identb")
    maskb = persist.tile([128, 128], BF16, tag="maskb")
    wqkT = persist.tile([128, NCC, 512], BF16, tag="wqkT")
    wvT = persist.tile([128, NCC, 256], BF16, tag="wvT")
    wpT = persist.tile([128, 2, C], BF16, tag="wpT")
    xT = [persist.tile([128, T], BF16, tag=f"xT{k}", name=f"xT{k}")
          for k in range(NCC)]
    qkT = [persist.tile([128, T], BF16, tag=f"qkT{m}", name=f"qkT{m}")
           for m in range(4)]
    # v strips: [k-tile 128, head, 64 dims + ones col] (col 64 = 1.0 for the
    # softmax denominator row of the PV matmul).
    vs = [persist.tile([128, HL, 65], BF16, tag=f"vs{i}", name=f"vs{i}")
          for i in range(NKT)]
    otj = [persist.tile([128, 2, 512], BF16, tag=f"otj{j}", name=f"otj{j}")
           for j in range(NQ)]

    # bf16 identity (for PSUM-accumulate mask adds) and the causal band mask:
    # maskb[p, q] = -30000 where q < p else 0 (additive, pre-exp).
    nc.gpsimd.memset(identb[:], 0.0)
    nc.gpsimd.affine_select(
        out=identb[:], in_=identb[:], compare_op=mybir.AluOpType.not_equal,
        fill=1.0, base=0, channel_multiplier=1, pattern=[[-1, 128]])
    nc.gpsimd.memset(maskb[:], 0.0)
    nc.gpsimd.affine_select(
        out=maskb[:], in_=maskb[:], compare_op=mybir.AluOpType.is_ge,
        fill=-30000.0, base=0, channel_multiplier=-1, pattern=[[1, 128]])
    for i in range(NKT):
        nc.gpsimd.memset(vs[i][:, :, 64:65], 1.0)

    # ---- loads (host already transposed + cast) ----
    # x chunks + qk weight chunks interleaved on the sync queue so phase A
    # can start after the first chunk; v/proj weights on the vector queue.
    for k in range(NCC):
        nc.sync.dma_start(wqkT[:, k, :], wqkt_d[k * 128:(k + 1) * 128, :])
        nc.sync.dma_start(xT[k][:], xt_d[k * 128:(k + 1) * 128, :])
    for k in range(NCC):
        nc.scalar.dma_start(wvT[:, k, :], wvt_d[k * 128:(k + 1) * 128, :])
    for c in range(2):
        nc.scalar.dma_start(wpT[:, c, :], wpt_d[c * 128:(c + 1) * 128, :])

    # ---- qk projection: qkT[m] = (wqk @ x.T) block rows, k-outer ----
    def emit_qk(ms):
        ps = {m: pp.tile([128, NQ, 512], F32, tag="quad", name=f"psqk{m}")
              for m in ms}
        for k in range(NCC):
            for m in ms:
                for n in range(NQ):
                    nc.tensor.matmul(
                        ps[m][:, n, :],
                        lhsT=wqkT[:, k, m * 128:(m + 1) * 128],
                        rhs=xT[k][:, n * 512:(n + 1) * 512],
                        start=(k == 0), stop=(k == NCC - 1))
        for m in ms:
            for n in range(NQ):
                if n % 2 == 0:
                    nc.vector.tensor_copy(qkT[m][:, n * 512:(n + 1) * 512],
                                          ps[m][:, n, :])
                else:
                    nc.scalar.copy(qkT[m][:, n * 512:(n + 1) * 512],
                                   ps[m][:, n, :])

    # ---- v projection, k-outer, 2 passes of 8 bank-sized accumulators ----
    def emit_v(i0):
        psv = [pp.tile([128, 4, 512], F32, tag="quad", name=f"psv{i0}_{h}")
               for h in range(2)]
        for k in range(NCC):
            for ii in range(8):
                i = i0 + ii
                nc.tensor.matmul(
                    psv[ii // 4][:, ii % 4, 0:256],
                    lhsT=xT[k][:, i * 128:(i + 1) * 128],
                    rhs=wvT[:, k, :],
                    start=(k == 0), stop=(k == NCC - 1))
        for ii in range(8):
            i = i0 + ii
            src = psv[ii // 4][:, ii % 4, 0:256].rearrange(
                "p (h d) -> p h d", h=HL)
            if ii % 2 == 0:
                nc.vector.tensor_copy(vs[i][:, :, 0:64], src)
            else:
                nc.scalar.copy(vs[i][:, :, 0:64], src)

    # ---- attention: j-outer, i-inner, all 4 heads per step ----
    def emit_attn():
        for j in range(NQ):
            otp = pp.tile([65, HL, 512], F32, tag="quad", name=f"otp{j}")
            for i in range(4 * j + 4):
                diag = (i // 4 == j)
                co = 128 * (i - 4 * j) if diag else 0
                psS = pp.tile([128, HL, 512], F32, tag="quad",
                              name=f"psS{j}_{i}")
                for p in range(2):
                    kt = qkT[2 + p]
                    qt = qkT[p]
                    for h in range(2):
                        pt0, pt1 = h * 64, h * 64 + 64
                        if diag and co < 384:
                            # one group per bank: the off-band part starts it,
                            # the band part joins, the mask matmul closes it
                            nc.tensor.matmul(
                                psS[:, 2 * p + h, co + 128:512],
                                lhsT=kt[pt0:pt1, i * 128:(i + 1) * 128],
                                rhs=qt[pt0:pt1,
                                       j * 512 + co + 128:(j + 1) * 512],
                                start=True, stop=False)
                            nc.tensor.matmul(
                                psS[:, 2 * p + h, co:co + 128],
                                lhsT=kt[pt0:pt1, i * 128:(i + 1) * 128],
                                rhs=qt[pt0:pt1,
                                       j * 512 + co:j * 512 + co + 128],
                                start=False, stop=False)
                        else:
                            nc.tensor.matmul(
                                psS[:, 2 * p + h, co:512],
                                lhsT=kt[pt0:pt1, i * 128:(i + 1) * 128],
                                rhs=qt[pt0:pt1, j * 512 + co:(j + 1) * 512],
                                start=True, stop=(not diag))
                if diag:
                    for c in range(HL):
                        nc.tensor.matmul(
                            psS[:, c, co:co + 128],
                            lhsT=identb[:],
                            rhs=maskb[:],
                            start=False, stop=True)
                pt = ptp.tile([128, HL, 512], BF16, tag="pt")
                nc.scalar.activation(pt[:, :, co:512], psS[:, :, co:512], Exp)
                for h in range(HL):
                    nc.tensor.matmul(
                        otp[:, h, co:512],
                        lhsT=vs[i][:, h, :],
                        rhs=pt[:, h, co:512],
                        start=(i == 0), stop=(i == 4 * j + 3))
            # normalize: free the otp PSUM banks with one copy, then
            # reciprocal on DVE and broadcast+mul on GpSimd.
            otC = octp.tile([65, HL, 512], F32, tag="otC")
            nc.vector.tensor_copy(otC[:], otp[:])
            li = rrp.tile([1, HL, 512], F32, tag="li")
            with nc.allow_low_precision(reason="recip of denominator row"):
                nc.vector.reciprocal(li[:], otC[64:65, :, :])
            lb = rrp.tile([64, HL, 512], F32, tag="lb")
            for h in range(HL):
                nc.gpsimd.partition_broadcast(lb[:, h, :], li[:, h, :])
                nc.gpsimd.tensor_mul(
                    otj[j][64 * (h % 2):64 * (h % 2) + 64, h // 2, :],
                    otC[0:64, h, :], lb[:, h, :])

    # ---- output projection ----
    def emit_op(j):
        for tbl in range(4):
            po = pout.tile([128, C], F16, tag="po")
            ps = pp.tile([128, HL, 512], F32, tag="quad", name=f"pso{j}_{tbl}")
            for c in range(2):
                for n2 in range(2):
                    nc.tensor.matmul(
                        ps[:, n2, :],
                        lhsT=otj[j][:, c, tbl * 128:(tbl + 1) * 128],
                        rhs=wpT[:, c, n2 * 512:(n2 + 1) * 512],
                        start=(c == 0), stop=(c == 1))
            for n2 in range(2):
                if n2 == 0:
                    nc.vector.tensor_copy(po[:, 0:512], ps[:, 0, :])
                else:
                    nc.scalar.copy(po[:, 512:1024], ps[:, 1, :])
            tb = 4 * j + tbl
            nc.sync.dma_start(p_d[tb * 128:(tb + 1) * 128, :], po[:])

    emit_qk((0, 2))
    emit_qk((1, 3))
    emit_v(0)
    emit_v(8)
    emit_attn()
    for j in range(NQ):
        emit_op(j)


_NC_CACHE = None


def _get_nc():
    global _NC_CACHE
    if _NC_CACHE is None:
        _pin_act_table()
        _NC_CACHE = build_nc()
    return _NC_CACHE


def make_in_maps(x, w_qkv, w_proj):
    x = np.asarray(x, np.float32)
    w_qkv = np.asarray(w_qkv, np.float32)
    w_proj = np.asarray(w_proj, np.float32)
    bf = ml_dtypes.bfloat16
    in_maps = []
    for c in range(8):
        b, g = divmod(c, 4)
        wq = w_qkv[g * 256:(g + 1) * 256] * 0.125  # fold 1/sqrt(Dh)
        wk = w_qkv[C + g * 256:C + (g + 1) * 256]
        wv = w_qkv[2 * C + g * 256:2 * C + (g + 1) * 256]
        wqk = np.concatenate([wq, wk], 0)  # [512, C]
        in_maps.append({
            "xt": np.ascontiguousarray(x[b].T).astype(bf),
            "wqkt": np.ascontiguousarray(wqk.T).astype(bf),
            "wvt": np.ascontiguousarray(wv.T).astype(bf),
            "wpt": np.ascontiguousarray(w_proj[:, g * 256:(g + 1) * 256].T).astype(bf),
        })
    return in_maps


def combine(results):
    return np.stack(
        [results[4 * b]["p"].astype(np.float32)
         + results[4 * b + 1]["p"].astype(np.float32)
         + results[4 * b + 2]["p"].astype(np.float32)
         + results[4 * b + 3]["p"].astype(np.float32)
         for b in range(2)], 0)


def kernel(x, w_qkv, w_proj):
    nc = _get_nc()
    res = bass_utils.run_bass_kernel_spmd(
        nc, make_in_maps(x, w_qkv, w_proj), core_ids=list(range(8)))
    return combine(res.results)


# revision 16
# speedup vs baseline: 1.2828x; 1.2828x over previous
"""Causal self-attention (B=2, T=2048, C=1024, H=16, Dh=64) on 8 trn2 NeuronCores.

Sharding: 2-way data-parallel over batch x 4-way tensor-parallel over heads.
Core c handles batch b=c//4 and heads 4g..4g+3 where g=c%4:
  - computes q,k (transposed layout) and v for its 4 heads,
  - causal flash-style attention per head entirely on-chip,
  - row-parallel output projection against w_proj[:, g*256:(g+1)*256],
  - returns the partial [T, C] projection in fp16; host sums the 4 partials.

Structure (v2, restructured for engine balance):
  A:  qk blocks m=0,2 (q/k of heads 0,1), k-outer so matmuls stream while
      the x chunks are still arriving from HBM.
  A2: qk blocks m=1,3.
  V:  v projection, k-outer, 16 row-tile accumulators live in PSUM.
  attn: j-outer (q-chunk), i-inner (k-tile); per (i,j) step all 4 heads:
      - scores as 2 row-tiled concurrent K=64 matmul pairs (heads share the
        PE array top/bottom half via tile_position inference),
      - causal mask added on the diagonal as a N=128 band matmul,
      - ONE exp per step over all 4 heads' scores ([128,4,512] PSUM AP),
      - PV with M=65 stationary (64 v-dims + ones row for the denominator).
      Normalization: copy otp->SBUF (frees PSUM fast), reciprocal on DVE,
      partition_broadcast + multiply on GpSimd.
  OP: output projection from otj (stationary) against wpT, fp16 out.

Softmax skips the max-subtraction (scores are O(1) here); the denominator
comes free as a 65th row in the PV matmul's stationary operand.
"""

import numpy as np
import ml_dtypes
from contextlib import ExitStack

import concourse.bass as bass
import concourse.tile as tile
from concourse import bacc, mybir, bass_utils

F32 = mybir.dt.float32
F16 = mybir.dt.float16
BF16 = mybir.dt.bfloat16

T = 2048
C = 1024
HL = 4  # local heads per core
DH = 64
NKT = T // 128  # 16 k-tiles
NQ = T // 512  # 4 q-chunks
NCC = C // 128  # 8 contraction chunks


def _pin_act_table():
    """Restrict the activation-table registry to the single set containing
    both Exp and Ln, so Exp interleaving never reloads tables."""
    import concourse.bacc as bacc_mod
    from concourse.hw_specs import get_activation_tables as real

    def only_combined(arch):
        t = real(arch)
        name = "natural_log_exp_and_others"
        if name in t:
            return {name: t[name]}
        return t

    bacc_mod.get_activation_tables = only_combined


def build_nc():
    nc = bacc.Bacc("TRN2", target_bir_lowering=False, debug=False)
    xt_d = nc.dram_tensor("xt", [C, T], BF16, kind="ExternalInput").ap()
    wqkt_d = nc.dram_tensor("wqkt", [C, 512], BF16, kind="ExternalInput").ap()
    wvt_d = nc.dram_tensor("wvt", [C, 256], BF16, kind="ExternalInput").ap()
    wpt_d = nc.dram_tensor("wpt", [256, C], BF16, kind="ExternalInput").ap()
    p_d = nc.dram_tensor("p", [T, C], F16, kind="ExternalOutput").ap()

    with tile.TileContext(nc) as tc:
        with ExitStack() as ctx:
            _body(ctx, tc, xt_d, wqkt_d, wvt_d, wpt_d, p_d)
    nc.compile()
    return nc


def _body(ctx, tc, xt_d, wqkt_d, wvt_d, wpt_d, p_d):
    nc = tc.nc
    Exp = mybir.ActivationFunctionType.Exp

    persist = ctx.enter_context(tc.tile_pool(name="persist", bufs=1))
    ptp = ctx.enter_context(tc.tile_pool(name="ptp", bufs=4))
    octp = ctx.enter_context(tc.tile_pool(name="octp", bufs=2))
    rrp = ctx.enter_context(tc.tile_pool(name="rrp", bufs=2))
    pout = ctx.enter_context(tc.tile_pool(name="pout", bufs=3))
    pp = ctx.enter_context(tc.tile_pool(name="pp", bufs=2, space="PSUM"))

    # ---- persistent SBUF tiles ----
    identb = persist.tile([128, 128], BF16, tag="identb")
    maskb = persist.tile([128, 128], BF16, tag="maskb")
    wqkT = persist.tile([128, NCC, 512], BF16, tag="wqkT")
    wvT = persist.tile([128, NCC, 256], BF16, tag="wvT")
    wpT = persist.tile([128, 2, C], BF16, tag="wpT")
    xT = [persist.tile([128, T], BF16, tag=f"xT{k}", name=f"xT{k}")
          for k in range(NCC)]
    qkT = [persist.tile([128, T], BF16, tag=f"qkT{m}", name=f"qkT{m}")
           for m in range(4)]
    # v strips: [k-tile 128, head, 64 dims + ones cols] (col 64 = 1.0 for the
    # softmax denominator row of the PV matmul; 65..127 pad the stationary to
    # 128 columns so FWL stays enabled and the weight load hides).
    vs = [persist.tile([128, HL, 128], BF16, tag=f"vs{i}", name=f"vs{i}")
          for i in range(NKT)]
    otj = [persist.tile([128, 2, 512], BF16, tag=f"otj{j}", name=f"otj{j}")
           for j in range(NQ)]

    # bf16 identity (for PSUM-accumulate mask adds) and the causal band mask:
    # maskb[p, q] = -30000 where q < p else 0 (additive, pre-exp).
    nc.gpsimd.memset(identb[:], 0.0)
    nc.gpsimd.affine_select(
        out=identb[:], in_=identb[:], compare_op=mybir.AluOpType.not_equal,
        fill=1.0, base=0, channel_multiplier=1, pattern=[[-1, 128]])
    nc.gpsimd.memset(maskb[:], 0.0)
    nc.gpsimd.affine_select(
        out=maskb[:], in_=maskb[:], compare_op=mybir.AluOpType.is_ge,
        fill=-30000.0, base=0, channel_multiplier=-1, pattern=[[1, 128]])
    for i in range(NKT):
        nc.gpsimd.memset(vs[i][:, :, 64:128], 1.0)

    # ---- loads (host already transposed + cast) ----
    # x chunks + qk weight chunks interleaved on the sync queue so phase A
    # can start after the first chunk; v/proj weights on the vector queue.
    for k in range(NCC):
        nc.sync.dma_start(wqkT[:, k, :], wqkt_d[k * 128:(k + 1) * 128, :])
        nc.sync.dma_start(xT[k][:], xt_d[k * 128:(k + 1) * 128, :])
    for k in range(NCC):
        nc.scalar.dma_start(wvT[:, k, :], wvt_d[k * 128:(k + 1) * 128, :])
    for c in range(2):
        nc.scalar.dma_start(wpT[:, c, :], wpt_d[c * 128:(c + 1) * 128, :])

    # ---- qk projection: qkT[m] = (wqk @ x.T) block rows, k-outer ----
    # PSUM layout: 1 quad tile (4 banks) for ms[0] + 2 duo tiles for ms[1].
    def emit_qk(ms):
        psA = pp.tile([128, NQ, 512], F32, tag="quad", bufs=1,
                      name=f"psqk{ms[0]}")
        psB = [pp.tile([128, 2, 512], F32, tag="duo", bufs=2,
                       name=f"psqk{ms[1]}_{t}") for t in range(2)]

        def tgt(m, n):
            return psA[:, n, :] if m == ms[0] else psB[n // 2][:, n % 2, :]

        for k in range(NCC):
            for m in ms:
                for n in range(NQ):
                    nc.tensor.matmul(
                        tgt(m, n),
                        lhsT=wqkT[:, k, m * 128:(m + 1) * 128],
                        rhs=xT[k][:, n * 512:(n + 1) * 512],
                        start=(k == 0), stop=(k == NCC - 1))
        for m in ms:
            for n in range(NQ):
                if n % 2 == 0:
                    nc.vector.tensor_copy(qkT[m][:, n * 512:(n + 1) * 512],
                                          tgt(m, n))
                else:
                    nc.scalar.copy(qkT[m][:, n * 512:(n + 1) * 512],
                                   tgt(m, n))

    # ---- v projection, k-outer, 2 passes of 8 bank-sized accumulators ----
    def emit_v(i0):
        pq = pp.tile([128, 4, 512], F32, tag="quad", bufs=1, name=f"psvq{i0}")
        pd = [pp.tile([128, 2, 512], F32, tag="duo", bufs=2,
                      name=f"psvd{i0}_{t}") for t in range(2)]

        def tgt(ii):
            if ii < 4:
                return pq[:, ii, 0:256]
            return pd[(ii - 4) // 2][:, ii % 2, 0:256]

        for k in range(NCC):
            for ii in range(8):
                i = i0 + ii
                nc.tensor.matmul(
                    tgt(ii),
                    lhsT=xT[k][:, i * 128:(i + 1) * 128],
                    rhs=wvT[:, k, :],
                    start=(k == 0), stop=(k == NCC - 1))
        for ii in range(8):
            i = i0 + ii
            src = tgt(ii).rearrange("p (h d) -> p h d", h=HL)
            if ii % 2 == 0:
                nc.vector.tensor_copy(vs[i][:, :, 0:64], src)
            else:
                nc.scalar.copy(vs[i][:, :, 0:64], src)

    # ---- attention: j-outer, i-inner, software-pipelined ----
    # Per step: scores for both head pairs (2-bank duo tiles, row-tiled
    # concurrent K=64 matmul pairs) + exp per pair; the PV of step i-1 is
    # emitted after step i's scores so the PE streams through the exp
    # latency instead of stalling on it.
    def emit_scores(j, i):
        diag = (i // 4 == j)
        co = 128 * (i - 4 * j) if diag else 0
        pts = []
        for p in range(2):
            kt = qkT[2 + p]
            qt = qkT[p]
            psS = pp.tile([128, 2, 512], F32, tag="duo", bufs=2,
                          name=f"psS{j}_{i}_{p}")
            for h in range(2):
                pt0, pt1 = h * 64, h * 64 + 64
                if diag and co < 384:
                    # one group per bank: the off-band part starts it, the
                    # band part joins, the mask matmul closes it
                    nc.tensor.matmul(
                        psS[:, h, co + 128:512],
                        lhsT=kt[pt0:pt1, i * 128:(i + 1) * 128],
                        rhs=qt[pt0:pt1, j * 512 + co + 128:(j + 1) * 512],
                        start=True, stop=False)
                    nc.tensor.matmul(
                        psS[:, h, co:co + 128],
                        lhsT=kt[pt0:pt1, i * 128:(i + 1) * 128],
                        rhs=qt[pt0:pt1, j * 512 + co:j * 512 + co + 128],
                        start=False, stop=False)
                else:
                    nc.tensor.matmul(
                        psS[:, h, co:512],
                        lhsT=kt[pt0:pt1, i * 128:(i + 1) * 128],
                        rhs=qt[pt0:pt1, j * 512 + co:(j + 1) * 512],
                        start=True, stop=(not diag))
            if diag:
                for h in range(2):
                    nc.tensor.matmul(
                        psS[:, h, co:co + 128],
                        lhsT=identb[:],
                        rhs=maskb[:],
                        start=False, stop=True)
            pt = ptp.tile([128, 2, 512], BF16, tag="pt", name=f"pt{j}_{i}_{p}")
            nc.scalar.activation(pt[:, :, co:512], psS[:, :, co:512], Exp)
            pts.append(pt)
        return pts, co

    def emit_pv(otp, j, i, pts, co):
        for h in range(HL):
            nc.tensor.matmul(
                otp[:, h, co:512],
                lhsT=vs[i][:, h, :],
                rhs=pts[h // 2][:, h % 2, co:512],
                start=(i == 0), stop=(i == 4 * j + 3))

    def emit_attn():
        for j in range(NQ):
            otp = pp.tile([128, HL, 512], F32, tag="quad", bufs=1,
                          name=f"otp{j}")
            prev = None
            for i in range(4 * j + 4):
                cur = emit_scores(j, i)
                if prev is not None:
                    emit_pv(otp, j, i - 1, *prev)
                prev = cur
            emit_pv(otp, j, 4 * j + 3, *prev)
            # normalize: free the otp PSUM banks with one copy, then
            # reciprocal on DVE and broadcast+mul on GpSimd.
            otC = octp.tile([65, HL, 512], F32, tag="otC")
            nc.vector.tensor_copy(otC[:], otp[0:65, :, :])
            li = rrp.tile([1, HL, 512], F32, tag="li")
            with nc.allow_low_precision(reason="recip of denominator row"):
                nc.vector.reciprocal(li[:], otC[64:65, :, :])
            lb = rrp.tile([64, HL, 512], F32, tag="lb")
            for h in range(HL):
                nc.gpsimd.partition_broadcast(lb[:, h, :], li[:, h, :])
                nc.gpsimd.tensor_mul(
                    otj[j][64 * (h % 2):64 * (h % 2) + 64, h // 2, :],
                    otC[0:64, h, :], lb[:, h, :])

    # ---- output projection ----
    def emit_op(j):
        for tbl in range(4):
            po = pout.tile([128, C], F16, tag="po")
            ps = pp.tile([128, 2, 512], F32, tag="duo", bufs=2,
                         name=f"pso{j}_{tbl}")
            for c in range(2):
                for n2 in range(2):
                    nc.tensor.matmul(
                        ps[:, n2, :],
                        lhsT=otj[j][:, c, tbl * 128:(tbl + 1) * 128],
                        rhs=wpT[:, c, n2 * 512:(n2 + 1) * 512],
                        start=(c == 0), stop=(c == 1))
            for n2 in range(2):
                if n2 == 0:
                    nc.vector.tensor_copy(po[:, 0:512], ps[:, 0, :])
                else:
                    nc.scalar.copy(po[:, 512:1024], ps[:, 1, :])
            tb = 4 * j + tbl
            nc.sync.dma_start(p_d[tb * 128:(tb + 1) * 128, :], po[:])

    emit_qk((0, 2))
    emit_qk((1, 3))
    emit_v(0)
    emit_v(8)
    emit_attn()
    for j in range(NQ):
        emit_op(j)


_NC_CACHE = None


def _get_nc():
    global _NC_CACHE
    if _NC_CACHE is None:
        _pin_act_table()
        _NC_CACHE = build_nc()
    return _NC_CACHE


def make_in_maps(x, w_qkv, w_proj):
    x = np.asarray(x, np.float32)
    w_qkv = np.asarray(w_qkv, np.float32)
    w_proj = np.asarray(w_proj, np.float32)
    bf = ml_dtypes.bfloat16
    in_maps = []
    for c in range(8):
        b, g = divmod(c, 4)
        wq = w_qkv[g * 256:(g + 1) * 256] * 0.125  # fold 1/sqrt(Dh)
        wk = w_qkv[C + g * 256:C + (g + 1) * 256]
        wv = w_qkv[2 * C + g * 256:2 * C + (g + 1) * 256]
        wqk = np.concatenate([wq, wk], 0)  # [512, C]
        in_maps.append({
            "xt": np.ascontiguousarray(x[b].T).astype(bf),
            "wqkt": np.ascontiguousarray(wqk.T).astype(bf),
            "wvt": np.ascontiguousarray(wv.T).astype(bf),
            "wpt": np.ascontiguousarray(w_proj[:, g * 256:(g + 1) * 256].T).astype(bf),
        })
    return in_maps


def combine(results):
    return np.stack(
        [results[4 * b]["p"].astype(np.float32)
         + results[4 * b + 1]["p"].astype(np.float32)
         + results[4 * b + 2]["p"].astype(np.float32)
         + results[4 * b + 3]["p"].astype(np.float32)
         for b in range(2)], 0)


def kernel(x, w_qkv, w_proj):
    nc = _get_nc()
    res = bass_utils.run_bass_kernel_spmd(
        nc, make_in_maps(x, w_qkv, w_proj), core_ids=list(range(8)))
    return combine(res.results)


# revision 18
# speedup vs baseline: 1.3305x; 1.0372x over previous
"""Causal self-attention (B=2, T=2048, C=1024, H=16, Dh=64) on 8 trn2 NeuronCores.

Sharding: 2-way data-parallel over batch x 4-way tensor-parallel over heads.
Core c handles batch b=c//4 and heads 4g..4g+3 where g=c%4:
  - computes q,k (transposed layout) and v for its 4 heads,
  - causal flash-style attention per head entirely on-chip,
  - row-parallel output projection against w_proj[:, g*256:(g+1)*256],
  - returns the partial [T, C] projection in fp16; host sums the 4 partials.

Structure (v2, restructured for engine balance):
  A:  qk blocks m=0,2 (q/k of heads 0,1), k-outer so matmuls stream while
      the x chunks are still arriving from HBM.
  A2: qk blocks m=1,3.
  V:  v projection, k-outer, 16 row-tile accumulators live in PSUM.
  attn: j-outer (q-chunk), i-inner (k-tile); per (i,j) step all 4 heads:
      - scores as 2 row-tiled concurrent K=64 matmul pairs (heads share the
        PE array top/bottom half via tile_position inference),
      - causal mask added on the diagonal as a N=128 band matmul,
      - ONE exp per step over all 4 heads' scores ([128,4,512] PSUM AP),
      - PV with M=65 stationary (64 v-dims + ones row for the denominator).
      Normalization: copy otp->SBUF (frees PSUM fast), reciprocal on DVE,
      partition_broadcast + multiply on GpSimd.
  OP: output projection from otj (stationary) against wpT, fp16 out.

Softmax skips the max-subtraction (scores are O(1) here); the denominator
comes free as a 65th row in the PV matmul's stationary operand.
"""

import numpy as np
import ml_dtypes
from contextlib import ExitStack

import concourse.bass as bass
import concourse.tile as tile
from concourse import bacc, mybir, bass_utils

F32 = mybir.dt.float32
F16 = mybir.dt.float16
BF16 = mybir.dt.bfloat16

T = 2048
C = 1024
HL = 4  # local heads per core
DH = 64
NKT = T // 128  # 16 k-tiles
NQ = T // 512  # 4 q-chunks
NCC = C // 128  # 8 contraction chunks


def _pin_act_table():
    """Restrict the activation-table registry to the single set containing
    both Exp and Ln, so Exp interleaving never reloads tables."""
    import concourse.bacc as bacc_mod
    from concourse.hw_specs import get_activation_tables as real

    def only_combined(arch):
        t = real(arch)
        name = "natural_log_exp_and_others"
        if name in t:
            return {name: t[name]}
        return t

    bacc_mod.get_activation_tables = only_combined


def build_nc():
    nc = bacc.Bacc("TRN2", target_bir_lowering=False, debug=False)
    xt_d = nc.dram_tensor("xt", [C, T], BF16, kind="ExternalInput").ap()
    wqkt_d = nc.dram_tensor("wqkt", [C, 512], BF16, kind="ExternalInput").ap()
    wvt_d = nc.dram_tensor("wvt", [C, 256], BF16, kind="ExternalInput").ap()
    wpt_d = nc.dram_tensor("wpt", [256, C], BF16, kind="ExternalInput").ap()
    p_d = nc.dram_tensor("p", [T, C], F16, kind="ExternalOutput").ap()

    with tile.TileContext(nc) as tc:
        with ExitStack() as ctx:
            _body(ctx, tc, xt_d, wqkt_d, wvt_d, wpt_d, p_d)
    nc.compile()
    return nc


def _body(ctx, tc, xt_d, wqkt_d, wvt_d, wpt_d, p_d):
    nc = tc.nc
    Exp = mybir.ActivationFunctionType.Exp

    persist = ctx.enter_context(tc.tile_pool(name="persist", bufs=1))
    ptp = ctx.enter_context(tc.tile_pool(name="ptp", bufs=4))
    octp = ctx.enter_context(tc.tile_pool(name="octp", bufs=2))
    rrp = ctx.enter_context(tc.tile_pool(name="rrp", bufs=2))
    pout = ctx.enter_context(tc.tile_pool(name="pout", bufs=3))
    pp = ctx.enter_context(tc.tile_pool(name="pp", bufs=2, space="PSUM"))

    # ---- persistent SBUF tiles ----
    identb = persist.tile([128, 128], BF16, tag="identb")
    maskb = persist.tile([128, 128], BF16, tag="maskb")
    wqkT = persist.tile([128, NCC, 512], BF16, tag="wqkT")
    wvT = persist.tile([128, NCC, 256], BF16, tag="wvT")
    wpT = persist.tile([128, 2, C], BF16, tag="wpT")
    xT = [persist.tile([128, T], BF16, tag=f"xT{k}", name=f"xT{k}")
          for k in range(NCC)]
    qkT = [persist.tile([128, T], BF16, tag=f"qkT{m}", name=f"qkT{m}")
           for m in range(4)]
    # v strips: [k-tile 128, head, 64 dims + ones cols] (col 64 = 1.0 for the
    # softmax denominator row of the PV matmul; 65..127 pad the stationary to
    # 128 columns so FWL stays enabled and the weight load hides).
    vs = [persist.tile([128, HL, 128], BF16, tag=f"vs{i}", name=f"vs{i}")
          for i in range(NKT)]
    otj = [persist.tile([128, 2, 512], BF16, tag=f"otj{j}", name=f"otj{j}")
           for j in range(NQ)]

    # bf16 identity (for PSUM-accumulate mask adds) and the causal band mask:
    # maskb[p, q] = -30000 where q < p else 0 (additive, pre-exp).
    nc.gpsimd.memset(identb[:], 0.0)
    nc.gpsimd.affine_select(
        out=identb[:], in_=identb[:], compare_op=mybir.AluOpType.not_equal,
        fill=1.0, base=0, channel_multiplier=1, pattern=[[-1, 128]])
    nc.gpsimd.memset(maskb[:], 0.0)
    nc.gpsimd.affine_select(
        out=maskb[:], in_=maskb[:], compare_op=mybir.AluOpType.is_ge,
        fill=-30000.0, base=0, channel_multiplier=-1, pattern=[[1, 128]])
    for i in range(NKT):
        nc.gpsimd.memset(vs[i][:, :, 64:128], 1.0)

    # ---- loads (host already transposed + cast) ----
    # x streams alone on the sync queue at full rate; all weights go on the
    # scalar engine's queue in parallel.
    for k in range(NCC):
        nc.sync.dma_start(xT[k][:], xt_d[k * 128:(k + 1) * 128, :])
    for k in range(NCC):
        nc.scalar.dma_start(wqkT[:, k, :], wqkt_d[k * 128:(k + 1) * 128, :])
    for k in range(NCC):
        nc.scalar.dma_start(wvT[:, k, :], wvt_d[k * 128:(k + 1) * 128, :])
    for c in range(2):
        nc.scalar.dma_start(wpT[:, c, :], wpt_d[c * 128:(c + 1) * 128, :])

    # ---- qk projection: qkT[m] = (wqk @ x.T) block rows, k-outer ----
    # PSUM layout: 1 quad tile (4 banks) for ms[0] + 2 duo tiles for ms[1].
    def emit_qk(ms):
        psA = pp.tile([128, NQ, 512], F32, tag="quad", bufs=1,
                      name=f"psqk{ms[0]}")
        psB = [pp.tile([128, 2, 512], F32, tag="duo", bufs=2,
                       name=f"psqk{ms[1]}_{t}") for t in range(2)]

        def tgt(m, n):
            return psA[:, n, :] if m == ms[0] else psB[n // 2][:, n % 2, :]

        for k in range(NCC):
            for m in ms:
                for n in range(NQ):
                    nc.tensor.matmul(
                        tgt(m, n),
                        lhsT=wqkT[:, k, m * 128:(m + 1) * 128],
                        rhs=xT[k][:, n * 512:(n + 1) * 512],
                        start=(k == 0), stop=(k == NCC - 1))
        for m in ms:
            for n in range(NQ):
                if n % 2 == 0:
                    nc.vector.tensor_copy(qkT[m][:, n * 512:(n + 1) * 512],
                                          tgt(m, n))
                else:
                    nc.scalar.copy(qkT[m][:, n * 512:(n + 1) * 512],
                                   tgt(m, n))

    # ---- v projection, k-outer, 2 passes of 8 bank-sized accumulators ----
    def emit_v(i0):
        pq = pp.tile([128, 4, 512], F32, tag="quad", bufs=1, name=f"psvq{i0}")
        pd = [pp.tile([128, 2, 512], F32, tag="duo", bufs=2,
                      name=f"psvd{i0}_{t}") for t in range(2)]

        def tgt(ii):
            if ii < 4:
                return pq[:, ii, 0:256]
            return pd[(ii - 4) // 2][:, ii % 2, 0:256]

        for k in range(NCC):
            for ii in range(8):
                i = i0 + ii
                nc.tensor.matmul(
                    tgt(ii),
                    lhsT=xT[k][:, i * 128:(i + 1) * 128],
                    rhs=wvT[:, k, :],
                    start=(k == 0), stop=(k == NCC - 1))
        for ii in range(8):
            i = i0 + ii
            src = tgt(ii).rearrange("p (h d) -> p h d", h=HL)
            if ii % 2 == 0:
                nc.vector.tensor_copy(vs[i][:, :, 0:64], src)
            else:
                nc.scalar.copy(vs[i][:, :, 0:64], src)

    # ---- attention: j-outer, i-inner, software-pipelined ----
    # Per step: scores for both head pairs (2-bank duo tiles, row-tiled
    # concurrent K=64 matmul pairs) + exp per pair; the PV of step i-1 is
    # emitted after step i's scores so the PE streams through the exp
    # latency instead of stalling on it.
    def emit_scores(j, i):
        diag = (i // 4 == j)
        co = 128 * (i - 4 * j) if diag else 0
        pts = []
        for p in range(2):
            kt = qkT[2 + p]
            qt = qkT[p]
            psS = pp.tile([128, 2, 512], F32, tag="duo", bufs=2,
                          name=f"psS{j}_{i}_{p}")
            for h in range(2):
                pt0, pt1 = h * 64, h * 64 + 64
                if diag and co < 384:
                    # one group per bank: the off-band part starts it, the
                    # band part joins, the mask matmul closes it
                    nc.tensor.matmul(
                        psS[:, h, co + 128:512],
                        lhsT=kt[pt0:pt1, i * 128:(i + 1) * 128],
                        rhs=qt[pt0:pt1, j * 512 + co + 128:(j + 1) * 512],
                        start=True, stop=False)
                    nc.tensor.matmul(
                        psS[:, h, co:co + 128],
                        lhsT=kt[pt0:pt1, i * 128:(i + 1) * 128],
                        rhs=qt[pt0:pt1, j * 512 + co:j * 512 + co + 128],
                        start=False, stop=False)
                else:
                    nc.tensor.matmul(
                        psS[:, h, co:512],
                        lhsT=kt[pt0:pt1, i * 128:(i + 1) * 128],
                        rhs=qt[pt0:pt1, j * 512 + co:(j + 1) * 512],
                        start=True, stop=(not diag))
            if diag:
                for h in range(2):
                    nc.tensor.matmul(
                        psS[:, h, co:co + 128],
                        lhsT=identb[:],
                        rhs=maskb[:],
                        start=False, stop=True)
            pt = ptp.tile([128, 2, 512], BF16, tag="pt", name=f"pt{j}_{i}_{p}")
            nc.scalar.activation(pt[:, :, co:512], psS[:, :, co:512], Exp)
            pts.append(pt)
        return pts, co

    def emit_pv(otp, j, i, pts, co):
        for h in range(HL):
            nc.tensor.matmul(
                otp[:, h, co:512],
                lhsT=vs[i][:, h, :],
                rhs=pts[h // 2][:, h % 2, co:512],
                start=(i == 0), stop=(i == 4 * j + 3))

    def emit_attn():
        for j in range(NQ):
            otp = pp.tile([128, HL, 512], F32, tag="quad", bufs=1,
                          name=f"otp{j}")
            prev = None
            for i in range(4 * j + 4):
                cur = emit_scores(j, i)
                if prev is not None:
                    emit_pv(otp, j, i - 1, *prev)
                prev = cur
            emit_pv(otp, j, 4 * j + 3, *prev)
            # normalize. otp rows 64..127 hold 64 broadcast copies of the
            # softmax denominator (the ones columns of vs), so a 64-lane
            # reciprocal needs no partition broadcast. Even heads multiply on
            # DVE straight out of PSUM; odd heads (partition-shifted into
            # otj's lower half) stage through SBUF and multiply on GpSimd.
            li = rrp.tile([64, HL, 512], F32, tag="li")
            with nc.allow_low_precision(reason="recip of denominator rows"):
                nc.vector.reciprocal(li[:], otp[64:128, :, :])
            otC = octp.tile([64, 2, 512], F32, tag="otC")
            nc.vector.tensor_copy(otC[:], otp[0:64, 1:4:2, :])
            for h in (0, 2):
                nc.vector.tensor_mul(
                    otj[j][0:64, h // 2, :], otp[0:64, h, :], li[:, h, :])
            for h in (1, 3):
                nc.gpsimd.tensor_mul(
                    otj[j][64:128, h // 2, :], otC[:, h // 2, :],
                    li[:, h, :])

    # ---- output projection ----
    def emit_op(j):
        for tbl in range(4):
            po = pout.tile([128, C], F16, tag="po")
            ps = pp.tile([128, 2, 512], F32, tag="duo", bufs=2,
                         name=f"pso{j}_{tbl}")
            for c in range(2):
                for n2 in range(2):
                    nc.tensor.matmul(
                        ps[:, n2, :],
                        lhsT=otj[j][:, c, tbl * 128:(tbl + 1) * 128],
                        rhs=wpT[:, c, n2 * 512:(n2 + 1) * 512],
                        start=(c == 0), stop=(c == 1))
            for n2 in range(2):
                if n2 == 0:
                    nc.vector.tensor_copy(po[:, 0:512], ps[:, 0, :])
                else:
                    nc.scalar.copy(po[:, 512:1024], ps[:, 1, :])
            tb = 4 * j + tbl
            nc.sync.dma_start(p_d[tb * 128:(tb + 1) * 128, :], po[:])

    emit_qk((0, 2))
    emit_qk((1, 3))
    emit_v(0)
    emit_v(8)
    emit_attn()
    for j in range(NQ):
        emit_op(j)


_NC_CACHE = None


def _get_nc():
    global _NC_CACHE
    if _NC_CACHE is None:
        _pin_act_table()
        _NC_CACHE = build_nc()
    return _NC_CACHE


def make_in_maps(x, w_qkv, w_proj):
    x = np.asarray(x, np.float32)
    w_qkv = np.asarray(w_qkv, np.float32)
    w_proj = np.asarray(w_proj, np.float32)
    bf = ml_dtypes.bfloat16
    in_maps = []
    for c in range(8):
        b, g = divmod(c, 4)
        wq = w_qkv[g * 256:(g + 1) * 256] * 0.125  # fold 1/sqrt(Dh)
        wk = w_qkv[C + g * 256:C + (g + 1) * 256]
        wv = w_qkv[2 * C + g * 256:2 * C + (g + 1) * 256]
        wqk = np.concatenate([wq, wk], 0)  # [512, C]
        in_maps.append({
            "xt": np.ascontiguousarray(x[b].T).astype(bf),
            "wqkt": np.ascontiguousarray(wqk.T).astype(bf),
            "wvt": np.ascontiguousarray(wv.T).astype(bf),
            "wpt": np.ascontiguousarray(w_proj[:, g * 256:(g + 1) * 256].T).astype(bf),
        })
    return in_maps


def combine(results):
    return np.stack(
        [results[4 * b]["p"].astype(np.float32)
         + results[4 * b + 1]["p"].astype(np.float32)
         + results[4 * b + 2]["p"].astype(np.float32)
         + results[4 * b + 3]["p"].astype(np.float32)
         for b in range(2)], 0)


def kernel(x, w_qkv, w_proj):
    nc = _get_nc()
    res = bass_utils.run_bass_kernel_spmd(
        nc, make_in_maps(x, w_qkv, w_proj), core_ids=list(range(8)))
    return combine(res.results)
